# revision 23
# baseline (speedup 1.0000x reference)
"""Multi-head attention (B=4, S=2048, D=512, H=8) on 8 TRN2 NeuronCores.

Sharding: the 8192 query rows (4 batches x 2048 seq) are split into 8 shards
of 1024 rows, one per core (core c -> batch c//2, query-half c%2).  Each core
computes full K/V projections for its batch (duplicated across the pair) and
the full output rows for its queries, so no collective is needed — the host
just concatenates the 8 output shards.

Per-core pipeline (all matmuls bf16, accumulation fp32):
  Q^T  [512,1024] = Wq^T.T @ xq^T   (head-pair-chunk layout, d_k on partitions)
  K^T  [512,2048],  V' [2048, 8*(64+1)]  (V plus a ones column per head ->
                                          softmax denominator for free)
  attention loop (qch outer, head-pair c inner, key-tile kt inner):
    sc[128k, h0|h1 512q each] : the two heads of a pair are emitted as
        back-to-back K=64 matmuls on disjoint PE row-groups
        (tile_position (0,0) / (64,0)) so they stream CONCURRENTLY —
        the scores phase runs at ~2x the serial rate.
    P = exp(0.125*sc + maskbias)   (one ScalarE activation per (c,kt))
    pv_h[65, 512] += V'_h.T @ P_h  (row 64 = softmax denominator)
    x_attn = pv[0:64] * bcast(1/pv[64])    (read PSUM directly)
  out[q,e] = x_attn^T.T @ Wo^T + bo'  (bo' = bo + Wo@bv folded host-side;
        wo + output DMA for the first query half overlap the second half's
        attention)
"""
import os
import sys

import numpy as np
import ml_dtypes

try:
    import concourse.bass as bass  # noqa: F401
except ImportError:  # fresh grading dir: fall back to the repo checkout
    for p in ("/root/.axon_site", "/root/.axon_site/_ro/trn_rl_repo",
              "/root/.axon_site/_ro/pypackages", "/opt/trn_rl_repo"):
        if os.path.isdir(p) and p not in sys.path:
            sys.path.insert(0, p)
    import concourse.bass as bass  # noqa: F401

import concourse.mybir as mybir
import concourse.tile as tile
from concourse import bacc
from concourse.bass_utils import run_bass_kernel_spmd

f32 = mybir.dt.float32
bf16 = mybir.dt.bfloat16
BF = ml_dtypes.bfloat16

B, S, D, H, DK = 4, 2048, 512, 8, 64
SQ = S // 2          # queries per core
NKT = S // 128       # 16 key tiles
NDC = D // 128       # 4 contraction chunks
PAIRS = H // 2       # 4 head pairs
EXP = mybir.ActivationFunctionType.Exp
MULT = mybir.AluOpType.mult
ADD = mybir.AluOpType.add

_NC_CACHE = None


def build_nc():
    global _NC_CACHE
    if _NC_CACHE is not None:
        return _NC_CACHE
    nc = bacc.Bacc("TRN2", target_bir_lowering=False, debug=False, num_devices=8)

    xall = nc.declare_dram_parameter("xall", [NDC, 128, SQ + 2 * S], bf16,
                                     isOutput=False)
    wall = nc.declare_dram_parameter("wall", [NDC, 128, 4 * D], bf16,
                                     isOutput=False)
    ball = nc.declare_dram_parameter("ball", [128, 2 * NDC + NKT], f32,
                                     isOutput=False)
    bvo = nc.declare_dram_parameter("bvo", [1, D], f32, isOutput=False)
    out = nc.declare_dram_parameter("out", [SQ, D], f32, isOutput=True)

    with tile.TileContext(nc) as tc:
        with (
            tc.tile_pool(name="const", bufs=1) as cp,
            tc.tile_pool(name="pt", bufs=3) as ptp,
            tc.tile_pool(name="den", bufs=2) as dnp,
            tc.tile_pool(name="rbc", bufs=2) as rbp,
            tc.tile_pool(name="ps_sc", bufs=2, space="PSUM") as ps_sc,
            tc.tile_pool(name="ps_pp", bufs=1, space="PSUM") as ps_pp,
            tc.tile_pool(name="ps_pv", bufs=3, space="PSUM") as ps_pv,
        ):
            # ---- constants / weights (fused DMAs; few dma_starts) ----
            wall_sb = cp.tile([128, NDC, 4 * D], bf16, tag="wall")
            xall_sb = cp.tile([128, NDC, SQ + 2 * S], bf16, tag="xall")
            ball_sb = cp.tile([128, 2 * NDC + NKT], f32, tag="ball")
            bvo_sb = cp.tile([1, D], f32, tag="bvo")
            # DMA issue order = need order:
            #  wq, xq-q0 (Q proj first half), wk, xk-k0 (KT tch0),
            #  small consts, wv, xv-k0 (V proj kt0-3), xk-k1, xv-rest,
            #  xk-rest, xq-q1, wo
            XQ0, XK0, XV0 = 0, SQ, SQ + S
            nc.sync.dma_start(
                wall_sb[:, :, 0:D],
                wall[:, :, 0:D].rearrange("c p n -> p c n"))
            nc.sync.dma_start(
                xall_sb[:, :, 0:SQ // 2],
                xall[:, :, 0:SQ // 2].rearrange("c p n -> p c n"))
            nc.sync.dma_start(
                wall_sb[:, :, D:2 * D],
                wall[:, :, D:2 * D].rearrange("c p n -> p c n"))
            nc.sync.dma_start(
                xall_sb[:, :, XK0:XK0 + 512],
                xall[:, :, XK0:XK0 + 512].rearrange("c p n -> p c n"))
            nc.sync.dma_start(ball_sb[:], ball[:])
            nc.sync.dma_start(bvo_sb[:], bvo[:])
            nc.sync.dma_start(
                wall_sb[:, :, 2 * D:3 * D],
                wall[:, :, 2 * D:3 * D].rearrange("c p n -> p c n"))
            nc.sync.dma_start(
                xall_sb[:, :, XV0:XV0 + 512],
                xall[:, :, XV0:XV0 + 512].rearrange("c p n -> p c n"))
            nc.sync.dma_start(
                xall_sb[:, :, XK0 + 512:XK0 + 1024],
                xall[:, :, XK0 + 512:XK0 + 1024].rearrange("c p n -> p c n"))
            nc.sync.dma_start(
                xall_sb[:, :, XV0 + 512:XV0 + 2048],
                xall[:, :, XV0 + 512:XV0 + 2048].rearrange("c p n -> p c n"))
            nc.sync.dma_start(
                xall_sb[:, :, XK0 + 1024:XK0 + 2048],
                xall[:, :, XK0 + 1024:XK0 + 2048].rearrange("c p n -> p c n"))
            nc.sync.dma_start(
                xall_sb[:, :, SQ // 2:SQ],
                xall[:, :, SQ // 2:SQ].rearrange("c p n -> p c n"))
            nc.sync.dma_start(
                wall_sb[:, :, 3 * D:4 * D],
                wall[:, :, 3 * D:4 * D].rearrange("c p n -> p c n"))
            wq_sb = wall_sb[:, :, 0 * D:1 * D]
            wk_sb = wall_sb[:, :, 1 * D:2 * D]
            wv_sb = wall_sb[:, :, 2 * D:3 * D]
            wo_sb = wall_sb[:, :, 3 * D:4 * D]
            bq_sb = ball_sb[:, 0:NDC]
            bk_sb = ball_sb[:, NDC:2 * NDC]
            maskb_sb = ball_sb[:, 2 * NDC:]
            bo_bc = cp.tile([128, D], f32, tag="bo_bc")
            nc.gpsimd.partition_broadcast(bo_bc[:], bvo_sb[0:1, 0:D])
            # preload the exp table set during the initial DMA window so the
            # first real activation doesn't pay the ~2.7us ACT_TABLE_LOAD
            warm = cp.tile([1, 16], f32, tag="warm")
            nc.vector.memset(warm[:, 0:8], 0.0)
            nc.scalar.activation(warm[:, 8:16], warm[:, 0:8], EXP,
                                 bias=0.0, scale=1.0)

            # ---- persistent activations ----
            QT_sb = cp.tile([128, PAIRS, SQ], bf16, tag="QT")
            KT_sb = cp.tile([128, PAIRS, S], bf16, tag="KT")
            # V' per head = [64 value dims | 64 replicated ones columns].
            # The ones block makes pv[64:128] hold 64 copies of the softmax
            # denominator -> normalize is a pure elementwise DVE chain (no
            # 1-partition reciprocal, no gpsimd partition broadcast).  M=128
            # costs the same as M=65: matmul time only depends on N.
            VP_sb = cp.tile([128, NKT, H * 128], bf16, tag="VP")
            XA_sb = cp.tile([128, PAIRS, SQ], bf16, tag="XA")
            ob_all = cp.tile([128, SQ // 128, D], f32, tag="ob_all")
            vp_ones = VP_sb[:].rearrange("p k (h c) -> p k h c", c=128)[:, :, :, 64:128]

            xq_c = [xall_sb[:, dc, 0:SQ] for dc in range(NDC)]
            xk_c = [xall_sb[:, dc, XK0:XK0 + S] for dc in range(NDC)]
            xv_c = [xall_sb[:, dc, XV0:XV0 + S] for dc in range(NDC)]

            # ---- emission helpers ----
            def proj_QT(c, qhalf):
                # one 512-wide query chunk of Q^T for pair c
                ps = ps_pp.tile([128, 512], f32, tag="pp")
                for dc in range(NDC):
                    nc.tensor.matmul(
                        ps[:],
                        wq_sb[:, dc, c * 128:(c + 1) * 128],
                        xq_c[dc][:, qhalf * 512:(qhalf + 1) * 512],
                        start=(dc == 0), stop=(dc == NDC - 1),
                    )
                nc.vector.tensor_scalar_add(
                    QT_sb[:, c, qhalf * 512:(qhalf + 1) * 512], ps[:],
                    bq_sb[:, c:c + 1],
                )

            def proj_KT(c, tch):
                # one 512-wide key chunk of K^T for pair c
                ps = ps_pp.tile([128, 512], f32, tag="pp")
                for dc in range(NDC):
                    nc.tensor.matmul(
                        ps[:],
                        wk_sb[:, dc, c * 128:(c + 1) * 128],
                        xk_c[dc][:, tch * 512:(tch + 1) * 512],
                        start=(dc == 0), stop=(dc == NDC - 1),
                    )
                nc.vector.tensor_scalar_add(
                    KT_sb[:, c, tch * 512:(tch + 1) * 512], ps[:],
                    bk_sb[:, c:c + 1],
                )

            def proj_V(kt):
                ps = ps_pp.tile([128, 512], f32, tag="pp")
                for dc in range(NDC):
                    nc.tensor.matmul(
                        ps[:],
                        xv_c[dc][:, kt * 128:(kt + 1) * 128],
                        wv_sb[:, dc, :],
                        start=(dc == 0), stop=(dc == NDC - 1),
                    )
                nc.vector.tensor_copy(
                    VP_sb[:, kt].rearrange("p (h c) -> p h c", c=128)[:, :, 0:64],
                    ps[:].rearrange("p (h c) -> p h c", c=64),
                )

            def wo_out(qt):
                ps = ps_pp.tile([128, 512], f32, tag="pp")
                for j in range(NDC):
                    nc.tensor.matmul(
                        ps[:],
                        XA_sb[:, j, qt * 128:(qt + 1) * 128],
                        wo_sb[:, j, :],
                        start=(j == 0), stop=(j == NDC - 1),
                    )
                nc.vector.tensor_tensor(
                    ob_all[:, qt, :], ps[:], bo_bc[:], ADD,
                )

            def wo_partial(qt, j):
                # single-j contribution to out[qt], accumulated in SBUF so
                # the last attention frame only gates one j-slice of wo
                ps = ps_pp.tile([128, 512], f32, tag="pp")
                nc.tensor.matmul(
                    ps[:],
                    XA_sb[:, j, qt * 128:(qt + 1) * 128],
                    wo_sb[:, j, :],
                    start=True, stop=True,
                )
                if j == 0:
                    nc.vector.tensor_tensor(
                        ob_all[:, qt, :], ps[:], bo_bc[:], ADD,
                    )
                else:
                    nc.vector.tensor_tensor(
                        ob_all[:, qt, :], ps[:], ob_all[:, qt, :], ADD,
                    )

            def emit_sc(qch, c, kt):
                # scores for the head pair: two K=64 matmuls on disjoint
                # PE row-groups (tile_position (0,0) and (64,0)) issued
                # back-to-back -> they execute concurrently.
                sc = ps_sc.tile([128, 1024], f32, tag="sc", name="sc")
                q0, q1 = qch * 512, (qch + 1) * 512
                nc.tensor.matmul(
                    sc[:, 0:512],
                    KT_sb[0:64, c, kt * 128:(kt + 1) * 128],
                    QT_sb[0:64, c, q0:q1],
                    start=True, stop=True,
                )
                nc.tensor.matmul(
                    sc[:, 512:1024],
                    KT_sb[64:128, c, kt * 128:(kt + 1) * 128],
                    QT_sb[64:128, c, q0:q1],
                    start=True, stop=True,
                )
                return sc

            def normalize(qch, c, half, pv):
                lo, hi = half * 64, (half + 1) * 64
                q0, q1 = qch * 512, (qch + 1) * 512
                # pv[64:128] = 64 replicated copies of the denominator.
                # Copy to SBUF first: the custom-DVE reciprocal must not
                # read PSUM directly (and GpSimd cannot access PSUM at all).
                den = dnp.tile([64, 512], f32, tag="den")
                nc.vector.tensor_copy(den[:], pv[64:128, :])
                rec = rbp.tile([64, 512], f32, tag="rec")
                nc.vector.reciprocal_approx_fast(out=rec[:], in_=den[:])
                nc.vector.tensor_tensor(
                    XA_sb[lo:hi, c, q0:q1], pv[0:64, :], rec[:], MULT,
                )

            # ---- schedule ----
            outr = out[:].rearrange("(q p) d -> p q d", p=128)

            its = [(qch, c, kt)
                   for qch in range(2) for c in range(PAIRS)
                   for kt in range(NKT)]

            # PE filler work emitted at (qch, c, kt) iteration start.
            fillers = {}

            def addf(key, fn):
                fillers.setdefault(key, []).append(fn)

            # V projection: VP[kt] consumed first at (0, 0, kt).
            # V(0) and V(1) go in the pre-phase; V(k) at iteration k-1.
            for k in range(2, NKT):
                addf((0, 0, k - 1), (lambda k=k: proj_V(k)))
            # K projection: KT(c, tch) consumed by sc(c, 4*tch..) which is
            # emitted with lookahead 2 -> must be emitted by iter 4*tch-3 of
            # frame c.  tch0 of frame c+1 at iter 12 of frame c.
            for c in range(PAIRS):
                for t in range(1, 4):
                    addf((0, c, 4 * t - 4), (lambda c=c, t=t: proj_KT(c, t)))
                if c + 1 < PAIRS:
                    addf((0, c, 11), (lambda c=c: proj_KT(c + 1, 0)))
            # Q projection for the second query half: needed from (1, 0, 0)
            # whose sc is emitted at (0, 3, 14).
            for c in range(PAIRS):
                addf((0, 1 + (c // 2), 2 + 4 * (c % 2)),
                     (lambda c=c: proj_QT(c, 1)))
            # wo for the first query half overlaps the second half's
            # attention; qt=c emitted inside frame (1, c).  The first-half
            # output DMA follows the last of those wo tiles.
            for c in range(PAIRS):
                addf((1, c, 2), (lambda c=c: wo_out(c)))
            addf((1, 3, 4),
                 lambda: nc.sync.dma_start(outr[:, 0:4], ob_all[:, 0:4]))
            # second-half wo: per-pair partial contributions, emitted as
            # fillers in the frame AFTER pair j's qch1 norm completes, so
            # the tail only contains the j=3 slices + per-qt DMA.
            for j in range(PAIRS - 1):
                for qt in range(4, SQ // 128):
                    addf((1, j + 1, 2 * (qt - 4) + 5),
                         (lambda qt=qt, j=j: wo_partial(qt, j)))

            # Q proj (first half) for pairs 1-3 as early fillers in frame c0
            for c in range(1, PAIRS):
                addf((0, 0, 2 * c - 1), (lambda c=c: proj_QT(c, 0)))
            # bulk of the V' ones block, staged off the critical path
            addf((0, 0, 2), lambda: nc.vector.memset(vp_ones[:, 6:11], 1.0))
            addf((0, 0, 5), lambda: nc.vector.memset(vp_ones[:, 11:NKT], 1.0))

            # pre-phase: only what the first scores need
            proj_QT(0, 0)
            proj_KT(0, 0)
            nc.vector.memset(vp_ones[:, 0:6], 1.0)
            proj_V(0)
            proj_V(1)

            sc_t = {}
            sc_t[its[0]] = emit_sc(*its[0])
            sc_t[its[1]] = emit_sc(*its[1])
            pv = {}
            for i, (qch, c, kt) in enumerate(its):
                for fn in fillers.get((qch, c, kt), []):
                    fn()
                if kt == 0:
                    pv[0] = ps_pv.tile([128, 512], f32, tag="pv",
                                       name=f"pv{qch}{c}h0")
                    pv[1] = ps_pv.tile([128, 512], f32, tag="pv",
                                       name=f"pv{qch}{c}h1")
                if i + 2 < len(its):
                    sc_t[its[i + 2]] = emit_sc(*its[i + 2])
                sc = sc_t.pop((qch, c, kt))
                pt = ptp.tile([128, 1024], bf16, tag="pt")
                nc.scalar.activation(
                    pt[:], sc[:], EXP,
                    bias=maskb_sb[:, kt:kt + 1], scale=0.125,
                )
                for half in range(2):
                    h = 2 * c + half
                    nc.tensor.matmul(
                        pv[half][:],
                        VP_sb[:, kt, h * 128:(h + 1) * 128],
                        pt[:, half * 512:(half + 1) * 512],
                        start=(kt == 0), stop=(kt == NKT - 1),
                    )
                if kt == NKT - 1:
                    normalize(qch, c, 0, pv[0])
                    normalize(qch, c, 1, pv[1])

            # tail: only the last pair's contribution to the second-half wo
            for qt in range(4, SQ // 128):
                wo_partial(qt, PAIRS - 1)
                nc.sync.dma_start(outr[:, qt:qt + 1], ob_all[:, qt:qt + 1])

    nc.finalize()
    _NC_CACHE = nc
    return nc


def make_in_maps(query, key, value, mask, Wq, bq, Wk, bk, Wv, bv, Wo, bo):
    query = np.asarray(query, np.float32)
    key = np.asarray(key, np.float32)
    value = np.asarray(value, np.float32)
    mask = np.asarray(mask)

    def wprep(W):
        return np.ascontiguousarray(
            np.asarray(W, np.float32).T.reshape(NDC, 128, D)
        ).astype(BF)

    wall_a = np.ascontiguousarray(np.concatenate(
        [wprep(Wq), wprep(Wk), wprep(Wv), wprep(Wo)], axis=2))
    bq_a = np.asarray(bq, np.float32).reshape(NDC, 128).T
    bk_a = np.asarray(bk, np.float32).reshape(NDC, 128).T
    # bv folds through the (normalized) attention into the output bias:
    # out = (P v_raw / den) Wo^T + (bo + Wo @ bv)
    bo2 = (np.asarray(bo, np.float32)
           + np.asarray(Wo, np.float32) @ np.asarray(bv, np.float32))
    bvo_a = np.ascontiguousarray(bo2.reshape(1, D))

    kT = key.transpose(0, 2, 1)    # [B, D, S]
    vT = value.transpose(0, 2, 1)
    qT = query.transpose(0, 2, 1)

    in_maps = []
    for core in range(8):
        b, qh = core // 2, core % 2
        xq_a = qT[b][:, qh * SQ:(qh + 1) * SQ].reshape(NDC, 128, SQ)
        xk_a = kT[b].reshape(NDC, 128, S)
        xv_a = vT[b].reshape(NDC, 128, S)
        xall_a = np.ascontiguousarray(
            np.concatenate([xq_a, xk_a, xv_a], axis=2)).astype(BF)
        mb = np.where(mask[b, 0] == 0, np.float32(-1e9), np.float32(0.0))
        mb = mb.reshape(NKT, 128).T
        ball_a = np.ascontiguousarray(
            np.concatenate([bq_a, bk_a, mb], axis=1)).astype(np.float32)
        in_maps.append({
            "xall": xall_a, "wall": wall_a, "ball": ball_a, "bvo": bvo_a,
        })
    return in_maps


def assemble_output(results):
    full = np.empty((B, S, D), np.float32)
    for core in range(8):
        b, qh = core // 2, core % 2
        full[b, qh * SQ:(qh + 1) * SQ, :] = results[core]["out"]
    return full


def kernel(**inputs):
    nc = build_nc()
    in_maps = make_in_maps(**inputs)
    res = run_bass_kernel_spmd(nc, in_maps, list(range(8))).results
    return assemble_output(res)


# revision 26
# speedup vs baseline: 1.0497x; 1.0497x over previous
"""Multi-head attention (B=4, S=2048, D=512, H=8) on 8 TRN2 NeuronCores.

Sharding: the 8192 query rows (4 batches x 2048 seq) are split into 8 shards
of 1024 rows, one per core (core c -> batch c//2, query-half c%2).  Each core
computes full K/V projections for its batch (duplicated across the pair) and
the full output rows for its queries, so no collective is needed — the host
just concatenates the 8 output shards.

Per-core pipeline (all matmuls bf16, accumulation fp32):
  Q^T  [512,1024] = Wq^T.T @ xq^T   (head-pair-chunk layout, d_k on partitions)
  K^T  [512,2048],  V' [2048, 8*(64+1)]  (V plus a ones column per head ->
                                          softmax denominator for free)
  attention loop (qch outer, head-pair c inner, key-tile kt inner):
    sc[128k, h0|h1 512q each] : the two heads of a pair are emitted as
        back-to-back K=64 matmuls on disjoint PE row-groups
        (tile_position (0,0) / (64,0)) so they stream CONCURRENTLY —
        the scores phase runs at ~2x the serial rate.
    P = exp(0.125*sc + maskbias)   (one ScalarE activation per (c,kt))
    pv_h[65, 512] += V'_h.T @ P_h  (row 64 = softmax denominator)
    x_attn = pv[0:64] * bcast(1/pv[64])    (read PSUM directly)
  out[q,e] = x_attn^T.T @ Wo^T + bo'  (bo' = bo + Wo@bv folded host-side;
        wo + output DMA for the first query half overlap the second half's
        attention)
"""
import os
import sys

import numpy as np
import ml_dtypes

try:
    import concourse.bass as bass  # noqa: F401
except ImportError:  # fresh grading dir: fall back to the repo checkout
    for p in ("/root/.axon_site", "/root/.axon_site/_ro/trn_rl_repo",
              "/root/.axon_site/_ro/pypackages", "/opt/trn_rl_repo"):
        if os.path.isdir(p) and p not in sys.path:
            sys.path.insert(0, p)
    import concourse.bass as bass  # noqa: F401

import concourse.mybir as mybir
import concourse.tile as tile
from concourse import bacc
from concourse.bass_utils import run_bass_kernel_spmd

f32 = mybir.dt.float32
bf16 = mybir.dt.bfloat16
BF = ml_dtypes.bfloat16

B, S, D, H, DK = 4, 2048, 512, 8, 64
SQ = S // 2          # queries per core
NKT = S // 128       # 16 key tiles
NDC = D // 128       # 4 contraction chunks
PAIRS = H // 2       # 4 head pairs
EXP = mybir.ActivationFunctionType.Exp
MULT = mybir.AluOpType.mult
ADD = mybir.AluOpType.add

_NC_CACHE = None


def build_nc():
    global _NC_CACHE
    if _NC_CACHE is not None:
        return _NC_CACHE
    nc = bacc.Bacc("TRN2", target_bir_lowering=False, debug=False, num_devices=8)

    xall = nc.declare_dram_parameter("xall", [NDC, 128, SQ + 2 * S], bf16,
                                     isOutput=False)
    wall = nc.declare_dram_parameter("wall", [NDC, 128, 4 * D], bf16,
                                     isOutput=False)
    ball = nc.declare_dram_parameter("ball", [128, 2 * NDC + NKT], f32,
                                     isOutput=False)
    bvo = nc.declare_dram_parameter("bvo", [1, D], f32, isOutput=False)
    out = nc.declare_dram_parameter("out", [SQ, D], f32, isOutput=True)

    with tile.TileContext(nc) as tc:
        with (
            tc.tile_pool(name="const", bufs=1) as cp,
            tc.tile_pool(name="pt", bufs=4) as ptp,
            tc.tile_pool(name="den", bufs=2) as dnp,
            tc.tile_pool(name="rbc", bufs=2) as rbp,
            tc.tile_pool(name="ps_sc", bufs=2, space="PSUM") as ps_sc,
            tc.tile_pool(name="ps_pp", bufs=2, space="PSUM") as ps_pp,
            tc.tile_pool(name="ps_pv", bufs=2, space="PSUM") as ps_pv,
        ):
            # ---- constants / weights (fused DMAs; few dma_starts) ----
            wall_sb = cp.tile([128, NDC, 4 * D], bf16, tag="wall")
            xall_sb = cp.tile([128, NDC, SQ + 2 * S], bf16, tag="xall")
            ball_sb = cp.tile([128, 2 * NDC + NKT], f32, tag="ball")
            bvo_sb = cp.tile([1, D], f32, tag="bvo")
            # DMA issue order = need order:
            #  wq, xq-q0 (Q proj first half), wk, xk-k0 (KT tch0),
            #  small consts, wv, xv-k0 (V proj kt0-3), xk-k1, xv-rest,
            #  xk-rest, xq-q1, wo
            XQ0, XK0, XV0 = 0, SQ, SQ + S
            nc.sync.dma_start(
                wall_sb[:, :, 0:D],
                wall[:, :, 0:D].rearrange("c p n -> p c n"))
            nc.sync.dma_start(
                xall_sb[:, :, 0:SQ // 2],
                xall[:, :, 0:SQ // 2].rearrange("c p n -> p c n"))
            nc.sync.dma_start(
                wall_sb[:, :, D:2 * D],
                wall[:, :, D:2 * D].rearrange("c p n -> p c n"))
            nc.sync.dma_start(
                xall_sb[:, :, XK0:XK0 + 512],
                xall[:, :, XK0:XK0 + 512].rearrange("c p n -> p c n"))
            nc.sync.dma_start(ball_sb[:], ball[:])
            nc.sync.dma_start(bvo_sb[:], bvo[:])
            nc.sync.dma_start(
                wall_sb[:, :, 2 * D:3 * D],
                wall[:, :, 2 * D:3 * D].rearrange("c p n -> p c n"))
            nc.sync.dma_start(
                xall_sb[:, :, XV0:XV0 + 512],
                xall[:, :, XV0:XV0 + 512].rearrange("c p n -> p c n"))
            nc.sync.dma_start(
                xall_sb[:, :, XK0 + 512:XK0 + 1024],
                xall[:, :, XK0 + 512:XK0 + 1024].rearrange("c p n -> p c n"))
            nc.sync.dma_start(
                xall_sb[:, :, XV0 + 512:XV0 + 2048],
                xall[:, :, XV0 + 512:XV0 + 2048].rearrange("c p n -> p c n"))
            nc.sync.dma_start(
                xall_sb[:, :, XK0 + 1024:XK0 + 2048],
                xall[:, :, XK0 + 1024:XK0 + 2048].rearrange("c p n -> p c n"))
            nc.sync.dma_start(
                xall_sb[:, :, SQ // 2:SQ],
                xall[:, :, SQ // 2:SQ].rearrange("c p n -> p c n"))
            nc.sync.dma_start(
                wall_sb[:, :, 3 * D:4 * D],
                wall[:, :, 3 * D:4 * D].rearrange("c p n -> p c n"))
            wq_sb = wall_sb[:, :, 0 * D:1 * D]
            wk_sb = wall_sb[:, :, 1 * D:2 * D]
            wv_sb = wall_sb[:, :, 2 * D:3 * D]
            wo_sb = wall_sb[:, :, 3 * D:4 * D]
            bq_sb = ball_sb[:, 0:NDC]
            bk_sb = ball_sb[:, NDC:2 * NDC]
            maskb_sb = ball_sb[:, 2 * NDC:]
            bo_bc = cp.tile([128, D], f32, tag="bo_bc")
            nc.gpsimd.partition_broadcast(bo_bc[:], bvo_sb[0:1, 0:D])
            # preload the exp table set during the initial DMA window so the
            # first real activation doesn't pay the ~2.7us ACT_TABLE_LOAD
            warm = cp.tile([1, 16], f32, tag="warm")
            nc.vector.memset(warm[:, 0:8], 0.0)
            nc.scalar.activation(warm[:, 8:16], warm[:, 0:8], EXP,
                                 bias=0.0, scale=1.0)

            # ---- persistent activations ----
            QT_sb = cp.tile([128, PAIRS, SQ], bf16, tag="QT")
            KT_sb = cp.tile([128, PAIRS, S], bf16, tag="KT")
            # V' per head = [64 value dims | 64 replicated ones columns].
            # The ones block makes pv[64:128] hold 64 copies of the softmax
            # denominator -> normalize is a pure elementwise DVE chain (no
            # 1-partition reciprocal, no gpsimd partition broadcast).  M=128
            # costs the same as M=65: matmul time only depends on N.
            VP_sb = cp.tile([128, NKT, H * 128], bf16, tag="VP")
            XA_sb = cp.tile([128, PAIRS, SQ], bf16, tag="XA")
            ob_all = cp.tile([128, SQ // 128, D], f32, tag="ob_all")
            vp_ones = VP_sb[:].rearrange("p k (h c) -> p k h c", c=128)[:, :, :, 64:128]

            xq_c = [xall_sb[:, dc, 0:SQ] for dc in range(NDC)]
            xk_c = [xall_sb[:, dc, XK0:XK0 + S] for dc in range(NDC)]
            xv_c = [xall_sb[:, dc, XV0:XV0 + S] for dc in range(NDC)]

            # ---- emission helpers ----
            def proj_QT(c, qhalf):
                # one 512-wide query chunk of Q^T for pair c
                ps = ps_pp.tile([128, 512], f32, tag="pp")
                for dc in range(NDC):
                    nc.tensor.matmul(
                        ps[:],
                        wq_sb[:, dc, c * 128:(c + 1) * 128],
                        xq_c[dc][:, qhalf * 512:(qhalf + 1) * 512],
                        start=(dc == 0), stop=(dc == NDC - 1),
                    )
                nc.vector.tensor_scalar_add(
                    QT_sb[:, c, qhalf * 512:(qhalf + 1) * 512], ps[:],
                    bq_sb[:, c:c + 1],
                )

            def proj_KT(c, tch):
                # one 512-wide key chunk of K^T for pair c
                ps = ps_pp.tile([128, 512], f32, tag="pp")
                for dc in range(NDC):
                    nc.tensor.matmul(
                        ps[:],
                        wk_sb[:, dc, c * 128:(c + 1) * 128],
                        xk_c[dc][:, tch * 512:(tch + 1) * 512],
                        start=(dc == 0), stop=(dc == NDC - 1),
                    )
                nc.vector.tensor_scalar_add(
                    KT_sb[:, c, tch * 512:(tch + 1) * 512], ps[:],
                    bk_sb[:, c:c + 1],
                )

            def proj_V(kt):
                ps = ps_pp.tile([128, 512], f32, tag="pp")
                for dc in range(NDC):
                    nc.tensor.matmul(
                        ps[:],
                        xv_c[dc][:, kt * 128:(kt + 1) * 128],
                        wv_sb[:, dc, :],
                        start=(dc == 0), stop=(dc == NDC - 1),
                    )
                nc.vector.tensor_copy(
                    VP_sb[:, kt].rearrange("p (h c) -> p h c", c=128)[:, :, 0:64],
                    ps[:].rearrange("p (h c) -> p h c", c=64),
                )

            def wo_out(qt):
                ps = ps_pp.tile([128, 512], f32, tag="pp")
                for j in range(NDC):
                    nc.tensor.matmul(
                        ps[:],
                        XA_sb[:, j, qt * 128:(qt + 1) * 128],
                        wo_sb[:, j, :],
                        start=(j == 0), stop=(j == NDC - 1),
                    )
                nc.vector.tensor_tensor(
                    ob_all[:, qt, :], ps[:], bo_bc[:], ADD,
                )

            def wo_partial(qt, j):
                # single-j contribution to out[qt], accumulated in SBUF so
                # the last attention frame only gates one j-slice of wo
                ps = ps_pp.tile([128, 512], f32, tag="pp")
                nc.tensor.matmul(
                    ps[:],
                    XA_sb[:, j, qt * 128:(qt + 1) * 128],
                    wo_sb[:, j, :],
                    start=True, stop=True,
                )
                if j == 0:
                    nc.vector.tensor_tensor(
                        ob_all[:, qt, :], ps[:], bo_bc[:], ADD,
                    )
                else:
                    nc.vector.tensor_tensor(
                        ob_all[:, qt, :], ps[:], ob_all[:, qt, :], ADD,
                    )

            def emit_sc(qch, c, kt):
                # scores for the head pair: two K=64 matmuls on disjoint
                # PE row-groups (tile_position (0,0) and (64,0)) issued
                # back-to-back -> they execute concurrently.
                sc = ps_sc.tile([128, 1024], f32, tag="sc", name="sc")
                q0, q1 = qch * 512, (qch + 1) * 512
                nc.tensor.matmul(
                    sc[:, 0:512],
                    KT_sb[0:64, c, kt * 128:(kt + 1) * 128],
                    QT_sb[0:64, c, q0:q1],
                    start=True, stop=True,
                )
                nc.tensor.matmul(
                    sc[:, 512:1024],
                    KT_sb[64:128, c, kt * 128:(kt + 1) * 128],
                    QT_sb[64:128, c, q0:q1],
                    start=True, stop=True,
                )
                return sc

            def normalize(qch, c, half, pv):
                lo, hi = half * 64, (half + 1) * 64
                q0, q1 = qch * 512, (qch + 1) * 512
                # pv[64:128] = 64 replicated copies of the denominator.
                # Copy to SBUF first: the custom-DVE reciprocal must not
                # read PSUM directly (and GpSimd cannot access PSUM at all).
                den = dnp.tile([64, 512], f32, tag="den")
                nc.vector.tensor_copy(den[:], pv[64:128, :])
                rec = rbp.tile([64, 512], f32, tag="rec")
                nc.vector.reciprocal_approx_fast(out=rec[:], in_=den[:])
                nc.vector.tensor_tensor(
                    XA_sb[lo:hi, c, q0:q1], pv[0:64, :], rec[:], MULT,
                )

            # ---- schedule ----
            outr = out[:].rearrange("(q p) d -> p q d", p=128)

            its = [(qch, c, kt)
                   for qch in range(2) for c in range(PAIRS)
                   for kt in range(NKT)]

            # PE filler work emitted at (qch, c, kt) iteration start.
            fillers = {}

            def addf(key, fn):
                fillers.setdefault(key, []).append(fn)

            # V projection: VP[kt] consumed first at (0, 0, kt).
            # V(0) and V(1) go in the pre-phase; V(k) at iteration k-1.
            for k in range(2, NKT):
                addf((0, 0, k - 1), (lambda k=k: proj_V(k)))
            # K projection: KT(c, tch) consumed by sc(c, 4*tch..) which is
            # emitted with lookahead 2 -> must be emitted by iter 4*tch-3 of
            # frame c.  tch0 of frame c+1 at iter 12 of frame c.
            for c in range(PAIRS):
                for t in range(1, 4):
                    addf((0, c, 4 * t - 4), (lambda c=c, t=t: proj_KT(c, t)))
                if c + 1 < PAIRS:
                    addf((0, c, 11), (lambda c=c: proj_KT(c + 1, 0)))
            # Q projection for the second query half: needed from (1, 0, 0)
            # whose sc is emitted at (0, 3, 14).
            for c in range(PAIRS):
                addf((0, 1 + (c // 2), 2 + 4 * (c % 2)),
                     (lambda c=c: proj_QT(c, 1)))
            # wo for the first query half overlaps the second half's
            # attention; qt=c emitted inside frame (1, c).  The first-half
            # output DMA follows the last of those wo tiles.
            for c in range(PAIRS):
                addf((1, c, 2), (lambda c=c: wo_out(c)))
            addf((1, 3, 4),
                 lambda: nc.sync.dma_start(outr[:, 0:4], ob_all[:, 0:4]))
            # second-half wo: per-pair partial contributions, emitted as
            # fillers in the frame AFTER pair j's qch1 norm completes, so
            # the tail only contains the j=3 slices + per-qt DMA.
            for j in range(PAIRS - 1):
                for qt in range(4, SQ // 128):
                    addf((1, j + 1, 2 * (qt - 4) + 5),
                         (lambda qt=qt, j=j: wo_partial(qt, j)))

            # Q proj (first half) for pairs 1-3 as early fillers in frame c0
            for c in range(1, PAIRS):
                addf((0, 0, 2 * c - 1), (lambda c=c: proj_QT(c, 0)))
            # bulk of the V' ones block, staged off the critical path
            addf((0, 0, 2), lambda: nc.vector.memset(vp_ones[:, 6:11], 1.0))
            addf((0, 0, 5), lambda: nc.vector.memset(vp_ones[:, 11:NKT], 1.0))

            # pre-phase: only what the first scores need
            proj_QT(0, 0)
            proj_KT(0, 0)
            nc.vector.memset(vp_ones[:, 0:6], 1.0)
            proj_V(0)
            proj_V(1)

            sc_t = {}
            sc_t[its[0]] = emit_sc(*its[0])
            sc_t[its[1]] = emit_sc(*its[1])
            pv = {}
            pt_t = {}

            def pv_mm(c, half, kt):
                nc.tensor.matmul(
                    pv[half][:],
                    VP_sb[:, kt, (2 * c + half) * 128:(2 * c + half + 1) * 128],
                    pt_t[kt][:, half * 512:(half + 1) * 512],
                    start=(kt == 0), stop=(kt == NKT - 1),
                )

            for i, (qch, c, kt) in enumerate(its):
                for fn in fillers.get((qch, c, kt), []):
                    fn()
                if kt == 0:
                    pv[0] = ps_pv.tile([128, 512], f32, tag="pv",
                                       name=f"pv{qch}{c}h0")
                    pv[1] = ps_pv.tile([128, 512], f32, tag="pv",
                                       name=f"pv{qch}{c}h1")
                if i + 2 < len(its):
                    sc_t[its[i + 2]] = emit_sc(*its[i + 2])
                sc = sc_t.pop((qch, c, kt))
                pt = ptp.tile([128, 1024], bf16, tag="pt")
                pt_t[kt] = pt
                nc.scalar.activation(
                    pt[:], sc[:], EXP,
                    bias=maskb_sb[:, kt:kt + 1], scale=0.125,
                )
                # h0 accumulates immediately; h1 lags 2 key-tiles so the
                # next frame's first h1 matmul never races the previous
                # frame's normalize chain for the pv slot.
                pv_mm(c, 0, kt)
                if kt >= 2:
                    pv_mm(c, 1, kt - 2)
                if kt == NKT - 1:
                    normalize(qch, c, 0, pv[0])
                    pv_mm(c, 1, NKT - 2)
                    pv_mm(c, 1, NKT - 1)
                    normalize(qch, c, 1, pv[1])

            # tail: only the last pair's contribution to the second-half wo
            for qt in range(4, SQ // 128):
                wo_partial(qt, PAIRS - 1)
                nc.sync.dma_start(outr[:, qt:qt + 1], ob_all[:, qt:qt + 1])

    nc.finalize()
    _NC_CACHE = nc
    return nc


def make_in_maps(query, key, value, mask, Wq, bq, Wk, bk, Wv, bv, Wo, bo):
    query = np.asarray(query, np.float32)
    key = np.asarray(key, np.float32)
    value = np.asarray(value, np.float32)
    mask = np.asarray(mask)

    def wprep(W):
        return np.ascontiguousarray(
            np.asarray(W, np.float32).T.reshape(NDC, 128, D)
        ).astype(BF)

    wall_a = np.ascontiguousarray(np.concatenate(
        [wprep(Wq), wprep(Wk), wprep(Wv), wprep(Wo)], axis=2))
    bq_a = np.asarray(bq, np.float32).reshape(NDC, 128).T
    bk_a = np.asarray(bk, np.float32).reshape(NDC, 128).T
    # bv folds through the (normalized) attention into the output bias:
    # out = (P v_raw / den) Wo^T + (bo + Wo @ bv)
    bo2 = (np.asarray(bo, np.float32)
           + np.asarray(Wo, np.float32) @ np.asarray(bv, np.float32))
    bvo_a = np.ascontiguousarray(bo2.reshape(1, D))

    kT = key.transpose(0, 2, 1)    # [B, D, S]
    vT = value.transpose(0, 2, 1)
    qT = query.transpose(0, 2, 1)

    in_maps = []
    for core in range(8):
        b, qh = core // 2, core % 2
        xq_a = qT[b][:, qh * SQ:(qh + 1) * SQ].reshape(NDC, 128, SQ)
        xk_a = kT[b].reshape(NDC, 128, S)
        xv_a = vT[b].reshape(NDC, 128, S)
        xall_a = np.ascontiguousarray(
            np.concatenate([xq_a, xk_a, xv_a], axis=2)).astype(BF)
        mb = np.where(mask[b, 0] == 0, np.float32(-1e9), np.float32(0.0))
        mb = mb.reshape(NKT, 128).T
        ball_a = np.ascontiguousarray(
            np.concatenate([bq_a, bk_a, mb], axis=1)).astype(np.float32)
        in_maps.append({
            "xall": xall_a, "wall": wall_a, "ball": ball_a, "bvo": bvo_a,
        })
    return in_maps


def assemble_output(results):
    full = np.empty((B, S, D), np.float32)
    for core in range(8):
        b, qh = core // 2, core % 2
        full[b, qh * SQ:(qh + 1) * SQ, :] = results[core]["out"]
    return full


def kernel(**inputs):
    nc = build_nc()
    in_maps = make_in_maps(**inputs)
    res = run_bass_kernel_spmd(nc, in_maps, list(range(8))).results
    return assemble_output(res)
